# revision 30
# baseline (speedup 1.0000x reference)
"""Trainium2 Bass kernel: Encoder_HieStackedCorr (UnCorrVmat_Detail, t_method='uncorr').

Math (per batch b):
    W1 = wn(U1_v, U1_g); W2 = wn(U2_v, U2_g)
    R = relu(V @ W1.T + b1)          [N, LR]
    L = relu(V @ W2.T + b2)          [N, LR]
    d[n] = L[n] . R[n];  dr = 1/sqrt(d + eps)
    s[m] = (N + 1 - dr[m] * (t . R[m])) / N,   t = sum_n dr[n] L[n,:]
    v = s @ V ;  feat = v @ W_lin.T + b_lin ; out = batchnorm(feat)

The N x N correlation matrix is never materialized (mean-pool commutes with
the matmul).  L|R are produced directly in *natural* [n, l|r] layout by using
the V^T tiles as the stationary matmul operand, so every per-n vector
(d, dr, s) lives across 128 partitions: rsqrt/affine are tiny [128, 16]
column ops instead of [1, N] row ops, and no transposes or PSUM-drain copies
are needed.

Contractions over n (t and v) use zero-padded stationary tiles whose active
columns sit at 32-partition spacing, because compute engines may only read
partition offsets that are 32-aligned.

Sharding: data-parallel over batch, 4 batches per core on 8 cores.  The tiny
[32,256] linear + batchnorm epilogue (cross-core batch stats) runs on host.
"""

import os
import sys

import numpy as np

for _p in ("/opt/trn_rl_repo", "/root/.axon_site/_ro/trn_rl_repo"):
    if os.path.isdir(_p) and _p not in sys.path:
        sys.path.insert(0, _p)
        break

import ml_dtypes  # noqa: E402
import concourse.bass as bass  # noqa: E402
import concourse.bacc as bacc  # noqa: E402
import concourse.mybir as mybir  # noqa: E402
import concourse.tile as tile  # noqa: E402
from concourse.bass_utils import run_bass_kernel_spmd  # noqa: E402


def _ensure_ntff_hook():
    """Shim the missing ``antenv.axon_hooks`` registry so trace=True works."""
    import types

    try:
        from antenv.axon_hooks import get_axon_ntff_profile_hook  # noqa: F401
        return
    except ImportError:
        pass
    try:
        from trn_agent_boot.trn_boot import _ntff_profile_via_ctypes
        hook = _ntff_profile_via_ctypes("/opt/axon/libaxon_pjrt.so")
    except Exception:
        hook = None
    mod = types.ModuleType("antenv.axon_hooks")
    mod._hook = hook
    mod.get_axon_ntff_profile_hook = lambda: mod._hook
    mod.set_axon_ntff_profile_hook = lambda h: setattr(mod, "_hook", h)
    sys.modules["antenv.axon_hooks"] = mod


_ensure_ntff_hook()

# Problem constants (hardcoded).
B, N, D, LR, EMB = 32, 2048, 256, 64, 256
NCORES = 8
B_LOC = B // NCORES          # 4 batches per core
NT = N // 128                # 16 row-tiles per batch
EPS_DIAG = 1e-6
EPS_BN = 1e-5

F32 = mybir.dt.float32
BF16 = mybir.dt.bfloat16
AF = mybir.ActivationFunctionType
ALU = mybir.AluOpType

CONFIG = dict(trace=False)

_CACHE = {}


def _build():
    nc = bacc.Bacc("TRN2", target_bir_lowering=False, debug=False)

    vt_d = nc.dram_tensor("vt", [2, 128, B_LOC * N], BF16, kind="ExternalInput").ap()
    vn_d = nc.dram_tensor("vn", [128, NT * 2 * B_LOC * 128], BF16,
                          kind="ExternalInput").ap()
    w12_d = nc.dram_tensor("w12", [2, 128, 128], BF16, kind="ExternalInput").ap()
    b12_d = nc.dram_tensor("b12", [128, 1], F32, kind="ExternalInput").ap()
    out_d = nc.dram_tensor("vmean", [1, B_LOC * D], F32, kind="ExternalOutput").ap()

    with tile.TileContext(nc) as tc:
        with (
            tc.tile_pool(name="const", bufs=1) as cpool,
            tc.tile_pool(name="prod", bufs=2) as prpool,
            tc.tile_pool(name="ps_lr", bufs=3, space="PSUM") as ps_lr,
            tc.tile_pool(name="ps_sm", bufs=1, space="PSUM") as ps_sm,
            tc.tile_pool(name="ps_v", bufs=1, space="PSUM") as ps_v,
        ):
            # ---- big persistent tiles ----
            vt4 = cpool.tile([128, B_LOC * 2 * N], BF16)        # [b, c, n]
            v4 = cpool.tile([128, NT * 2 * B_LOC * 128], BF16)  # [j, c, b, d]
            LR4 = cpool.tile([128, NT * B_LOC * 128], BF16)     # [j, b, lr]
            d4 = cpool.tile([128, B_LOC * NT], F32)             # [b, j]
            sq4 = cpool.tile([128, B_LOC * NT], F32)
            drf4 = cpool.tile([128, B_LOC * NT], F32)
            u4 = cpool.tile([128, B_LOC * NT], F32)
            cs4 = cpool.tile([128, B_LOC * NT], F32)
            # zero-padded stationaries: active col 32*b of block j; separate
            # tiles per batch pair so late writes never conflict with the
            # earlier pair's matmul reads.
            drpA = cpool.tile([128, NT * 128], BF16)
            drpB = cpool.tile([128, NT * 128], BF16)
            sA = cpool.tile([128, NT * 128], BF16)
            sB = cpool.tile([128, NT * 128], BF16)
            t_sb = cpool.tile([1, B_LOC * LR], BF16)
            tb_sb = cpool.tile([128, B_LOC * LR], BF16)
            out32 = cpool.tile([128, D], F32)

            # ---- input DMAs first; weights lead, then vt0 in fine-grained
            # pieces so many queues work on the first-needed data in parallel.
            w12_sb = cpool.tile([128, 2 * 128], BF16)
            nc.sync.dma_start(
                w12_sb[:].rearrange("p (c m) -> p c m", c=2),
                w12_d.rearrange("c p m -> p c m"),
            )
            b12_sb = cpool.tile([128, 1], F32)
            nc.sync.dma_start(b12_sb[:], b12_d[:])

            def vt_load(b, c, lo, hi):
                nc.sync.dma_start(
                    vt4[:, (b * 2 + c) * N + lo:(b * 2 + c) * N + hi],
                    vt_d[c, :, b * N + lo:b * N + hi],
                )

            for lo, hi in ((0, 256), (256, 512), (512, 1024), (1024, 2048)):
                for c in range(2):
                    vt_load(0, c, lo, hi)
            for lo, hi in ((0, 1024), (1024, 2048)):
                for c in range(2):
                    vt_load(1, c, lo, hi)
            for b in (2, 3):
                for c in range(2):
                    vt_load(b, c, 0, 2048)
            # vn is needed only by the final v-matmuls; issue from the ACT
            # queue (second hwdge engine) while it is still idle.
            vn_q = NT * 2 * B_LOC * 128 // 4
            for q in range(4):
                nc.scalar.dma_start(
                    v4[:, q * vn_q:(q + 1) * vn_q],
                    vn_d[:, q * vn_q:(q + 1) * vn_q],
                )

            eps_sb = cpool.tile([128, 1], F32)
            nc.vector.memset(eps_sb[:], EPS_DIAG)
            ones_k1 = cpool.tile([1, 128], BF16)
            nc.vector.memset(ones_k1[:], 1.0)
            nc.gpsimd.memset(drpA[:], 0.0)
            nc.gpsimd.memset(drpB[:], 0.0)
            nc.gpsimd.memset(sA[:], 0.0)
            nc.gpsimd.memset(sB[:], 0.0)

            t_ps = ps_sm.tile([128, B_LOC * 128], F32, tag="tps")
            tb_ps = ps_sm.tile([128, B_LOC * LR], F32, tag="tbps")
            warm_ps = ps_sm.tile([128, 256], F32, tag="warm")
            v_ps = [ps_v.tile([128, 512], F32, tag=f"vps{h}", name=f"vps{h}")
                    for h in range(2)]

            LR4v = LR4[:].rearrange("p (j b l) -> p j b l", j=NT, b=B_LOC)
            drpv = [drpA[:].rearrange("p (j m) -> p j m", j=NT),
                    drpB[:].rearrange("p (j m) -> p j m", j=NT)]
            sv = [sA[:].rearrange("p (j m) -> p j m", j=NT),
                  sB[:].rearrange("p (j m) -> p j m", j=NT)]

            def emit_blocks(b):
                """L|R for batch b directly in natural [n, l|r] layout."""
                for g in range(4):
                    lr_ps = ps_lr.tile([128, 512], F32, tag="lrps")
                    for q in range(4):
                        j = g * 4 + q
                        for c in range(2):
                            nc.tensor.matmul(
                                lr_ps[:, q * 128:(q + 1) * 128],
                                vt4[:, (b * 2 + c) * N + j * 128:
                                    (b * 2 + c) * N + (j + 1) * 128],
                                w12_sb[:, c * 128:(c + 1) * 128],
                                start=(c == 0), stop=(c == 1),
                            )
                    dst = LR4v[:, g * 4:(g + 1) * 4, b, :]
                    srcv = lr_ps[:].rearrange("p (q l) -> p q l", q=4)
                    nc.scalar.activation(dst, srcv, AF.Relu,
                                         bias=b12_sb[:], scale=1.0)

            def emit_d(b):
                """diag -> dr (column layout) for batch b; dr lands in drp."""
                Lb = LR4v[:, :, b, 0:LR]
                Rb = LR4v[:, :, b, LR:128]
                pr = prpool.tile([128, NT * LR], BF16, tag="pr")
                prv = pr[:].rearrange("p (j l) -> p j l", j=NT)
                nc.vector.tensor_tensor(prv, Lb, Rb, ALU.mult)
                nc.vector.tensor_reduce(
                    d4[:, b * NT:(b + 1) * NT], prv,
                    mybir.AxisListType.X, ALU.add,
                )
                nc.scalar.activation(
                    sq4[:, b * NT:(b + 1) * NT], d4[:, b * NT:(b + 1) * NT],
                    AF.Sqrt, bias=eps_sb[:], scale=1.0,
                )
                nc.vector.reciprocal(
                    drf4[:, b * NT:(b + 1) * NT], sq4[:, b * NT:(b + 1) * NT]
                )
                nc.scalar.activation(
                    drpv[b // 2][:, :, 32 * b],
                    drf4[:, b * NT:(b + 1) * NT], AF.Copy,
                )

            def emit_t_chain(pair):
                """t for batch pair: rows 32*b of t_ps (zero-padded lhsT)."""
                drp = (drpA, drpB)[pair]
                for j in range(NT):
                    nc.tensor.matmul(
                        t_ps[:], drp[:, j * 128:(j + 1) * 128],
                        LR4[:, j * 512:(j + 1) * 512],
                        start=(j == 0), stop=(j == NT - 1),
                    )
                for b in (2 * pair, 2 * pair + 1):
                    nc.scalar.activation(
                        t_sb[0:1, b * LR:(b + 1) * LR],
                        t_ps[32 * b:32 * b + 1, b * 128:b * 128 + LR], AF.Copy,
                    )
                    nc.tensor.matmul(
                        tb_ps[:, b * LR:(b + 1) * LR], ones_k1[:],
                        t_sb[0:1, b * LR:(b + 1) * LR], start=True, stop=True,
                    )
                    nc.scalar.activation(
                        tb_sb[:, b * LR:(b + 1) * LR],
                        tb_ps[:, b * LR:(b + 1) * LR], AF.Copy,
                    )

            def emit_u(b):
                """u[m] = t . R[m,:] via broadcast multiply + segmented reduce."""
                Rb = LR4v[:, :, b, LR:128]
                tb = tb_sb[:, b * LR:(b + 1) * LR].rearrange(
                    "p (a l) -> p a l", a=1)
                in0, in1 = bass.broadcast_tensor_aps(Rb, tb)
                pr = prpool.tile([128, NT * LR], BF16, tag="pr")
                prv = pr[:].rearrange("p (j l) -> p j l", j=NT)
                nc.vector.tensor_tensor(prv, in0, in1, ALU.mult)
                nc.vector.tensor_reduce(
                    u4[:, b * NT:(b + 1) * NT], prv,
                    mybir.AxisListType.X, ALU.add,
                )

            def emit_cs(b):
                """s = ((N+1) - dr*u)/N, bf16, padded column 32*b."""
                nc.vector.scalar_tensor_tensor(
                    cs4[:, b * NT:(b + 1) * NT], u4[:, b * NT:(b + 1) * NT],
                    -1.0 / N, drf4[:, b * NT:(b + 1) * NT],
                    ALU.mult, ALU.mult,
                )
                nc.scalar.activation(
                    sv[b // 2][:, :, 32 * b], cs4[:, b * NT:(b + 1) * NT],
                    AF.Copy, bias=float(N + 1) / N, scale=1.0,
                )

            def emit_warmup():
                """Dummy chained matmuls to ramp the PE pstate while the
                first vt pieces stream in; result is never read."""
                for i in range(16):
                    nc.tensor.matmul(
                        warm_ps[:], w12_sb[:, 0:128], w12_sb[:],
                        start=(i == 0), stop=(i == 15),
                    )

            def emit_v(half):
                """v for batch pair `half`: b in {2h, 2h+1}, row 32*b."""
                v4v = v4[:].rearrange("p (j c b e) -> p j c b e",
                                      j=NT, c=2, b=B_LOC)
                s_t = (sA, sB)[half]
                # one accumulation group at a time per PSUM bank
                for c in range(2):
                    for j in range(NT):
                        nc.tensor.matmul(
                            v_ps[half][:, c * 256:(c + 1) * 256],
                            s_t[:, j * 128:(j + 1) * 128],
                            v4v[:, j, c, 2 * half:2 * half + 2, :],
                            start=(j == 0), stop=(j == NT - 1),
                        )
                for b in (2 * half, 2 * half + 1):
                    for c in range(2):
                        src = v_ps[half][32 * b:32 * b + 1,
                                         c * 256 + (b - 2 * half) * 128:
                                         c * 256 + (b - 2 * half + 1) * 128]
                        dst = out32[32 * b:32 * b + 1, c * 128:(c + 1) * 128]
                        if c == 0:
                            nc.scalar.activation(dst, src, AF.Copy)
                        else:
                            nc.vector.tensor_copy(dst, src)
                    nc.sync.dma_start(
                        out_d[0:1, b * D:(b + 1) * D],
                        out32[32 * b:32 * b + 1, :],
                    )

            # ---- emission order: keep PE busy while vector chains run ----
            emit_warmup()
            emit_blocks(0)
            emit_d(0)
            emit_blocks(1)
            emit_d(1)
            emit_blocks(2)
            emit_t_chain(0)
            emit_u(0)
            emit_cs(0)
            emit_u(1)
            emit_d(2)
            emit_cs(1)
            emit_blocks(3)
            emit_v(0)
            emit_d(3)
            emit_t_chain(1)
            emit_u(2)
            emit_cs(2)
            emit_u(3)
            emit_cs(3)
            emit_v(1)

    nc.compile()
    return nc


def _host_prep(inputs):
    """Weight-norm, transposes, casts; returns per-core input maps."""
    bf = ml_dtypes.bfloat16

    def wn(v, g):
        return v * (g / np.linalg.norm(v.astype(np.float64), axis=1)).astype(
            np.float32
        )[:, None]

    W1 = wn(np.asarray(inputs["U1_v"], np.float32), np.asarray(inputs["U1_g"], np.float32))
    W2 = wn(np.asarray(inputs["U2_v"], np.float32), np.asarray(inputs["U2_g"], np.float32))
    W12 = np.concatenate([W2, W1], axis=0)  # [128, 256]
    w12 = np.stack([np.ascontiguousarray(W12[:, :128].T),
                    np.ascontiguousarray(W12[:, 128:].T)]).astype(bf)
    b12 = np.concatenate([np.asarray(inputs["U2_b"], np.float32),
                          np.asarray(inputs["U1_b"], np.float32)]).reshape(128, 1)

    V = np.asarray(inputs["Vmat"], np.float32)  # [B, N, D]
    in_maps = []
    for k in range(NCORES):
        Vk = V[k * B_LOC:(k + 1) * B_LOC]  # [4, 2048, 256]
        vt = np.ascontiguousarray(Vk.transpose(2, 0, 1)).reshape(
            2, 128, B_LOC * N).astype(bf)
        vn = np.ascontiguousarray(
            Vk.reshape(B_LOC, NT, 128, 2, 128).transpose(2, 1, 3, 0, 4)
        ).reshape(128, NT * 2 * B_LOC * 128).astype(bf)
        in_maps.append({
            "vt": np.ascontiguousarray(vt),
            "vn": np.ascontiguousarray(vn),
            "w12": w12,
            "b12": b12,
        })
    return in_maps


def _epilogue(v_mean, inputs):
    """feat = v_mean @ W_lin.T + b_lin, then training-mode batchnorm."""
    W_lin = np.asarray(inputs["W_lin"], np.float32)
    b_lin = np.asarray(inputs["b_lin"], np.float32)
    gamma = np.asarray(inputs["gamma"], np.float32)
    beta = np.asarray(inputs["beta"], np.float32)
    feat = v_mean.astype(np.float32) @ W_lin.T + b_lin
    mu = feat.mean(axis=0)
    var = feat.var(axis=0)
    out = (feat - mu) / np.sqrt(var + EPS_BN) * gamma + beta
    return out.astype(np.float32)


def kernel(**inputs):
    if "nc" not in _CACHE:
        _CACHE["nc"] = _build()
    nc = _CACHE["nc"]
    in_maps = _host_prep(inputs)
    res = run_bass_kernel_spmd(
        nc, in_maps, core_ids=list(range(NCORES)), trace=CONFIG["trace"]
    )
    kernel.last_results = res
    v_mean = np.concatenate(
        [res.results[k]["vmean"].reshape(B_LOC, D) for k in range(NCORES)], axis=0
    )
    return _epilogue(v_mean, inputs)
